# revision 6
# baseline (speedup 1.0000x reference)
"""Multi-head attention (B=2, S=2048, D=1024, H=16) on 8 NeuronCores.

Sharding: core c -> batch b = c//4, head group g = c%4 (heads 4g..4g+3).
Megatron-style: Wq/Wk/Wv column-split (rows of the torch weight), Wo row-split
(columns of the torch weight).  Per-core output partials are summed on host.

Per-core device kernel (all shapes fp32):
  inputs : QT,KT,VT (1024,2048) = X[b].T ; WQT/WKT/WVT (1024,256) = W_g.T
           (WQT,BQ pre-scaled by 1/sqrt(dk)); WOT (256,1024) = Wo[:,g].T
           BQ/BK/BV (1,256)
  outputs: ATTN (4,2048,2048) softmax probs for this core's heads
           OUTP (2048,1024) partial of out @ Wo^T (no bo)

Pipeline per core:
  1. q^T,k^T,v^T = W^T.T @ X^T  (PE, contraction over D, bias folded in via
     a rank-1 ones matmul appended to each accumulation group)
  2. v^T -> v (natural [seq, dh]) via PE transposes
  3. per head: s_nat[i,j] (PE, two heads packed in the 128 partitions via
     row tile_position), e=exp(s) + row sums (ACT accum_out), r=1/sum (DVE),
     p = e*r (DVE tensor_scalar, per-partition scalar), DMA p -> ATTN.
  4. per head: s_T[j,i] (PE, swapped operands), e_T=exp (ACT),
     o_T[d,i] += v.T @ e_T (PE, col tile_position packs the two heads),
     normalized by r broadcast along the free dim (r replicated to all
     partitions via a DMA broadcast from a DRAM bounce).
  5. out[i,:] += o_norm^T.T @ WOT (PE), DMA -> OUTP.
"""

import os
import numpy as np

S = 2048
D = 1024
H = 16
DK = 64
B = 2
HC = 4  # heads per core
P = 128
N_CORES = 8
F32 = None  # set after import

_cached = {}


def _build():
    import concourse.bass as bass
    import concourse.mybir as mybir
    import concourse.tile as tile
    from concourse import bacc
    from concourse.masks import make_identity

    f32 = mybir.dt.float32
    nc = bacc.Bacc(
        "TRN2", target_bir_lowering=False, debug=False, num_devices=N_CORES
    )

    QT = nc.dram_tensor("qt_in", (D, S), f32, kind="ExternalInput").ap()
    KT = nc.dram_tensor("kt_in", (D, S), f32, kind="ExternalInput").ap()
    VT = nc.dram_tensor("vt_in", (D, S), f32, kind="ExternalInput").ap()
    WQT = nc.dram_tensor("wqt", (D, HC * DK), f32, kind="ExternalInput").ap()
    WKT = nc.dram_tensor("wkt", (D, HC * DK), f32, kind="ExternalInput").ap()
    WVT = nc.dram_tensor("wvt", (D, HC * DK), f32, kind="ExternalInput").ap()
    WOT = nc.dram_tensor("wot", (HC * DK, D), f32, kind="ExternalInput").ap()
    BQ = nc.dram_tensor("bq_in", (1, HC * DK), f32, kind="ExternalInput").ap()
    BK = nc.dram_tensor("bk_in", (1, HC * DK), f32, kind="ExternalInput").ap()
    BV = nc.dram_tensor("bv_in", (1, HC * DK), f32, kind="ExternalInput").ap()
    ATTN = nc.dram_tensor("attn_out", (HC, S, S), f32, kind="ExternalOutput").ap()
    OUTP = nc.dram_tensor("out_partial", (S, D), f32, kind="ExternalOutput").ap()

    IT = S // P       # 16 i tiles
    JT = S // P       # 16 j tiles
    NKT = D // P      # 8 contraction tiles
    DH = HC * DK      # 256 local head dims

    with tile.TileContext(nc) as tc:
        from contextlib import ExitStack

        with ExitStack() as ctx:
            # ---------------- pools ----------------
            consts = ctx.enter_context(tc.tile_pool(name="consts", bufs=1))
            persist = ctx.enter_context(tc.tile_pool(name="persist", bufs=1))
            psum = ctx.enter_context(tc.tile_pool(name="psum", bufs=2, space="PSUM"))
            dram = ctx.enter_context(tc.tile_pool(name="dram", bufs=2, space="DRAM"))

            identity = consts.tile([P, P], f32)
            make_identity(nc, identity)
            ones_row = consts.tile([1, 512], f32)
            nc.vector.memset(ones_row, 1.0)

            # persistent SBUF tensors
            qT = persist.tile([P, 2, S], f32)   # [:, p, :] = heads 2p,2p+1
            kT = persist.tile([P, 2, S], f32)
            v_sb = persist.tile([P, JT, DH], f32)   # [j within tile, jt, dh]
            o_sb = persist.tile([P, 2, S], f32)     # normalized o^T per pair
            wot_sb = persist.tile([P, 2, D], f32)   # [:, p, :] = WOT rows
            r_mat = persist.tile([P, HC, IT], f32)  # 1/rowsum, [:, h, it]

            nc.sync.dma_start(wot_sb[:, 0, :], WOT[0:P, :])
            nc.sync.dma_start(wot_sb[:, 1, :], WOT[P : 2 * P, :])

            bias_sb = consts.tile([1, 3, DH], f32)
            nc.sync.dma_start(bias_sb[:, 0, :], BQ[:, :])
            nc.sync.dma_start(bias_sb[:, 1, :], BK[:, :])
            nc.sync.dma_start(bias_sb[:, 2, :], BV[:, :])

            # ---------------- phase 1: projections ----------------
            with tc.tile_pool(name="projw", bufs=1) as wpool, tc.tile_pool(
                name="xin", bufs=3
            ) as xin:
                w_sb = wpool.tile([P, 3, NKT, DH], f32)  # wq, wk, wv tiles
                for wi, W in enumerate((WQT, WKT, WVT)):
                    for kt in range(NKT):
                        nc.sync.dma_start(
                            w_sb[:, wi, kt, :], W[kt * P : (kt + 1) * P, :]
                        )

                vT_tmp = wpool.tile([P, 2, S], f32)  # v^T before transpose

                for wi, (X, dst) in enumerate(((QT, qT), (KT, kT), (VT, vT_tmp))):
                    for ihalf in range(2):
                        ps = [
                            psum.tile([P, 1024], f32, tag="nat", name=f"ps{m}")
                            for m in range(2)
                        ]
                        for kt in range(NKT):
                            xt = xin.tile([P, 1024], f32, tag="xin")
                            nc.sync.dma_start(
                                xt,
                                X[
                                    kt * P : (kt + 1) * P,
                                    ihalf * 1024 : (ihalf + 1) * 1024,
                                ],
                            )
                            for m in range(2):
                                for ib in range(2):
                                    nc.tensor.matmul(
                                        ps[m][:, ib * 512 : (ib + 1) * 512],
                                        w_sb[:, wi, kt, m * P : (m + 1) * P],
                                        xt[:, ib * 512 : (ib + 1) * 512],
                                        start=(kt == 0),
                                        stop=False,
                                    )
                        for m in range(2):
                            for ib in range(2):
                                nc.tensor.matmul(
                                    ps[m][:, ib * 512 : (ib + 1) * 512],
                                    bias_sb[0:1, wi, m * P : (m + 1) * P],
                                    ones_row[0:1, :],
                                    start=False,
                                    stop=True,
                                )
                            nc.vector.tensor_copy(
                                dst[:, m, ihalf * 1024 : (ihalf + 1) * 1024],
                                ps[m][:, :],
                            )

                # v^T -> v natural via PE transposes
                for m in range(2):
                    for jc in range(JT):
                        tp = psum.tile([P, P], f32, tag="st")
                        nc.tensor.transpose(
                            tp, vT_tmp[:, m, jc * P : (jc + 1) * P], identity
                        )
                        nc.vector.tensor_copy(
                            v_sb[:, jc, m * P : (m + 1) * P], tp
                        )

            # ---------------- phase 2/3: attention ----------------
            with tc.tile_pool(name="enat", bufs=3) as enat, tc.tile_pool(
                name="pnat", bufs=3
            ) as pnat, tc.tile_pool(name="etp", bufs=4) as etp, tc.tile_pool(
                name="rbc", bufs=3
            ) as rbc_pool, tc.tile_pool(name="smallp", bufs=4) as smallp, tc.tile_pool(
                name="outp", bufs=3
            ) as outp:
                rbc = {}
                for pair in range(2):
                    qTp = qT[:, pair, :]
                    kTp = kT[:, pair, :]
                    # ---- natural-layout scores + softmax + ATTN output ----
                    for e in range(2):
                        h = 2 * pair + e
                        lo, hi = e * DK, (e + 1) * DK
                        for it in range(IT):
                            e_nat = enat.tile([P, S], f32, tag="enat")
                            sums = smallp.tile([P, 2], f32, tag="sums")
                            for jh in range(2):
                                psn = psum.tile([P, 1024], f32, tag="nat")
                                for jb in range(2):
                                    j0 = jh * 1024 + jb * 512
                                    nc.tensor.matmul(
                                        psn[:, jb * 512 : (jb + 1) * 512],
                                        qTp[lo:hi, it * P : (it + 1) * P],
                                        kTp[lo:hi, j0 : j0 + 512],
                                        start=True,
                                        stop=True,
                                    )
                                nc.scalar.activation(
                                    e_nat[:, jh * 1024 : (jh + 1) * 1024],
                                    psn[:, :],
                                    mybir.ActivationFunctionType.Exp,
                                    accum_out=sums[:, jh : jh + 1],
                                )
                            stot = smallp.tile([P, 1], f32, tag="stot")
                            nc.vector.tensor_reduce(
                                stot,
                                sums,
                                axis=mybir.AxisListType.X,
                                op=mybir.AluOpType.add,
                            )
                            nc.vector.reciprocal(r_mat[:, h, it : it + 1], stot)
                            p_t = pnat.tile([P, S], f32, tag="pnat")
                            nc.vector.tensor_scalar_mul(
                                p_t, e_nat, r_mat[:, h, it : it + 1]
                            )
                            nc.sync.dma_start(
                                ATTN[h, it * P : (it + 1) * P, :], p_t
                            )
                        # r broadcast: r_mat[:,h,:] -> dram (transposed) -> all partitions
                        rt_d = dram.tile([S], f32)
                        rt_write = bass.AP(
                            tensor=rt_d.tensor,
                            offset=rt_d.offset,
                            ap=[[1, P], [P, IT]],
                        )
                        with nc.allow_non_contiguous_dma(
                            reason="2k-element transposed scatter of row sums"
                        ):
                            nc.gpsimd.dma_start(rt_write, r_mat[:, h, :])
                        rbc[h] = rbc_pool.tile(
                            [P, S], f32, tag="rbc", name=f"rbc{h}"
                        )
                        rt_bcast = bass.AP(
                            tensor=rt_d.tensor,
                            offset=rt_d.offset,
                            ap=[[0, P], [1, S]],
                        )
                        nc.gpsimd.dma_start(rbc[h], rt_bcast)

                    # ---- transposed scores + AV ----
                    for iq in range(4):
                        av = psum.tile([P, 512], f32, tag="av")
                        for jt in range(JT):
                            for e in range(2):
                                lo, hi = e * DK, (e + 1) * DK
                                st = psum.tile([P, 512], f32, tag="st")
                                nc.tensor.matmul(
                                    st,
                                    kTp[lo:hi, jt * P : (jt + 1) * P],
                                    qTp[lo:hi, iq * 512 : (iq + 1) * 512],
                                    start=True,
                                    stop=True,
                                )
                                et = etp.tile([P, 512], f32, tag="etp")
                                nc.scalar.activation(
                                    et, st, mybir.ActivationFunctionType.Exp
                                )
                                nc.tensor.matmul(
                                    av[lo:hi, :],
                                    v_sb[:, jt, pair * P + lo : pair * P + hi],
                                    et,
                                    start=(jt == 0),
                                    stop=(jt == JT - 1),
                                    tile_position=(0, lo),
                                )
                        for e in range(2):
                            h = 2 * pair + e
                            lo, hi = e * DK, (e + 1) * DK
                            nc.vector.tensor_tensor(
                                o_sb[lo:hi, pair, iq * 512 : (iq + 1) * 512],
                                av[lo:hi, :],
                                rbc[h][lo:hi, iq * 512 : (iq + 1) * 512],
                                mybir.AluOpType.mult,
                            )

                # ---------------- phase 4: output projection ----------------
                for it in range(IT):
                    out_t = outp.tile([P, D], f32, tag="outp")
                    for nb in range(2):
                        po = psum.tile([P, 512], f32, tag="st")
                        for pair in range(2):
                            nc.tensor.matmul(
                                po,
                                o_sb[:, pair, it * P : (it + 1) * P],
                                wot_sb[:, pair, nb * 512 : (nb + 1) * 512],
                                start=(pair == 0),
                                stop=(pair == 1),
                            )
                        nc.vector.tensor_copy(
                            out_t[:, nb * 512 : (nb + 1) * 512], po
                        )
                    nc.sync.dma_start(OUTP[it * P : (it + 1) * P, :], out_t)

    nc.compile()
    return nc


def _get_nc():
    if "nc" not in _cached:
        _cached["nc"] = _build()
    return _cached["nc"]


def _prep_in_maps(Q, K, V, Wq, bq, Wk, bk, Wv, bv, Wo, bo):
    f4 = np.float32
    Q = np.asarray(Q, f4)
    K = np.asarray(K, f4)
    V = np.asarray(V, f4)
    Wq = np.asarray(Wq, f4)
    Wk = np.asarray(Wk, f4)
    Wv = np.asarray(Wv, f4)
    Wo = np.asarray(Wo, f4)
    bq = np.asarray(bq, f4)
    bk = np.asarray(bk, f4)
    bv = np.asarray(bv, f4)
    bo = np.asarray(bo, f4)

    scale = f4(1.0 / np.sqrt(DK))
    QTb = [np.ascontiguousarray(Q[b].T) for b in range(B)]
    KTb = [np.ascontiguousarray(K[b].T) for b in range(B)]
    VTb = [np.ascontiguousarray(V[b].T) for b in range(B)]

    in_maps = []
    for c in range(N_CORES):
        b, g = c // 4, c % 4
        rows = slice(g * HC * DK, (g + 1) * HC * DK)
        in_maps.append(
            {
                "qt_in": QTb[b],
                "kt_in": KTb[b],
                "vt_in": VTb[b],
                "wqt": np.ascontiguousarray(Wq[rows].T) * scale,
                "wkt": np.ascontiguousarray(Wk[rows].T),
                "wvt": np.ascontiguousarray(Wv[rows].T),
                "wot": np.ascontiguousarray(Wo[:, rows].T),
                "bq_in": (bq[rows] * scale).reshape(1, -1),
                "bk_in": bk[rows].reshape(1, -1),
                "bv_in": bv[rows].reshape(1, -1),
            }
        )
    return in_maps, bo


def _assemble(results, bo):
    f4 = np.float32
    attn = np.empty((B, H, S, S), f4)
    out = np.tile(np.broadcast_to(bo, (S, D)).astype(f4)[None], (B, 1, 1))
    for c in range(N_CORES):
        b, g = c // 4, c % 4
        attn[b, g * HC : (g + 1) * HC] = results[c]["attn_out"]
        out[b] += results[c]["out_partial"]
    return (out, attn)


def kernel(Q, K, V, Wq, bq, Wk, bk, Wv, bv, Wo, bo):
    from concourse import bass_utils

    in_maps, bo = _prep_in_maps(Q, K, V, Wq, bq, Wk, bk, Wv, bv, Wo, bo)
    nc = _get_nc()
    res = bass_utils.run_bass_kernel_spmd(
        nc, in_maps, core_ids=list(range(N_CORES))
    )
    _cached["last_results"] = res
    return _assemble(res.results, bo)


# revision 11
# speedup vs baseline: 1.5822x; 1.5822x over previous
"""Multi-head attention (B=2, S=2048, D=1024, H=16) on 8 NeuronCores.

Sharding: core c -> batch b = c//4, head group g = c%4 (heads 4g..4g+3).
Megatron-style: Wq/Wk/Wv column-split (rows of the torch weight), Wo row-split
(columns of the torch weight).  Per-core output partials are summed on host.

Per-core device kernel (all shapes fp32):
  inputs : QT,KT,VT (1024,2048) = X[b].T ; WQT/WKT/WVT (1024,256) = W_g.T
           (WQT,BQ pre-scaled by 1/sqrt(dk)); WOT (256,1024) = Wo[:,g].T
           BQ/BK/BV (1,256)
  outputs: ATTN (4,2048,2048) softmax probs for this core's heads
           OUTP (2048,1024) partial of out @ Wo^T (no bo)

Pipeline per core:
  1. q^T,k^T,v^T = W^T.T @ X^T  (PE, contraction over D, bias folded in via
     a rank-1 ones matmul appended to each accumulation group)
  2. v^T -> v (natural [seq, dh]) via PE transposes
  3. per head: s_nat[i,j] (PE, two heads packed in the 128 partitions via
     row tile_position), e=exp(s) + row sums (ACT accum_out), r=1/sum (DVE),
     p = e*r (DVE tensor_scalar, per-partition scalar), DMA p -> ATTN.
  4. per head: s_T[j,i] (PE, swapped operands), e_T=exp (ACT),
     o_T[d,i] += v.T @ e_T (PE, col tile_position packs the two heads),
     normalized by r broadcast along the free dim (r replicated to all
     partitions via a DMA broadcast from a DRAM bounce).
  5. out[i,:] += o_norm^T.T @ WOT (PE), DMA -> OUTP.
"""

import os
import numpy as np

S = 2048
D = 1024
H = 16
DK = 64
B = 2
HC = 4  # heads per core
P = 128
N_CORES = 8
F32 = None  # set after import

_cached = {}


def _build():
    import concourse.bass as bass
    import concourse.mybir as mybir
    import concourse.tile as tile
    from concourse import bacc
    from concourse.masks import make_identity

    f32 = mybir.dt.float32
    f32r = mybir.dt.float32r

    nc = bacc.Bacc(
        "TRN2", target_bir_lowering=False, debug=False, num_devices=N_CORES
    )

    QT = nc.dram_tensor("qt_in", (D, S), f32r, kind="ExternalInput").ap()
    KT = nc.dram_tensor("kt_in", (D, S), f32r, kind="ExternalInput").ap()
    VT = nc.dram_tensor("vt_in", (D, S), f32r, kind="ExternalInput").ap()
    WQT = nc.dram_tensor("wqt", (D, HC * DK), f32r, kind="ExternalInput").ap()
    WKT = nc.dram_tensor("wkt", (D, HC * DK), f32r, kind="ExternalInput").ap()
    WVT = nc.dram_tensor("wvt", (D, HC * DK), f32r, kind="ExternalInput").ap()
    WOT = nc.dram_tensor("wot", (HC * DK, D), f32r, kind="ExternalInput").ap()
    BQ = nc.dram_tensor("bq_in", (1, HC * DK), f32r, kind="ExternalInput").ap()
    BK = nc.dram_tensor("bk_in", (1, HC * DK), f32r, kind="ExternalInput").ap()
    BV = nc.dram_tensor("bv_in", (1, HC * DK), f32r, kind="ExternalInput").ap()
    ONES = nc.dram_tensor("ones_in", (1, 512), f32r, kind="ExternalInput").ap()
    ATTN = nc.dram_tensor("attn_out", (HC, S, S), f32, kind="ExternalOutput").ap()
    OUTP = nc.dram_tensor("out_partial", (S, D), f32, kind="ExternalOutput").ap()

    IT = S // P       # 16 i tiles
    JT = S // P       # 16 j tiles
    NKT = D // P      # 8 contraction tiles
    DH = HC * DK      # 256 local head dims

    mm = nc.tensor.matmul

    with tile.TileContext(nc) as tc:
        from contextlib import ExitStack

        with ExitStack() as ctx:
            # ---------------- pools ----------------
            consts = ctx.enter_context(tc.tile_pool(name="consts", bufs=1))
            persist = ctx.enter_context(tc.tile_pool(name="persist", bufs=1))
            psum = ctx.enter_context(tc.tile_pool(name="psum", bufs=2, space="PSUM"))
            dram = ctx.enter_context(tc.tile_pool(name="dram", bufs=2, space="DRAM"))

            identity = consts.tile([P, P], f32)
            make_identity(nc, identity)
            ones_row = consts.tile([1, 512], f32r)
            nc.sync.dma_start(ones_row, ONES[:, :])

            # persistent SBUF tensors
            qT = persist.tile([P, 2, S], f32r)   # [:, p, :] = heads 2p,2p+1
            kT = persist.tile([P, 2, S], f32r)
            v_sb = persist.tile([P, JT, DH], f32r)   # [j within tile, jt, dh]
            o_sb = persist.tile([64, HC, S], f32r)   # normalized o^T per head
            wot_sb = persist.tile([64, HC, D], f32r)  # [:, h, :] = WOT rows
            r_mat = persist.tile([P, HC, IT], f32)  # 1/rowsum, [:, h, it]

            for h in range(HC):
                nc.sync.dma_start(wot_sb[:, h, :], WOT[h * DK : (h + 1) * DK, :])

            bias_sb = consts.tile([1, 3, DH], f32r)
            nc.sync.dma_start(bias_sb[:, 0, :], BQ[:, :])
            nc.sync.dma_start(bias_sb[:, 1, :], BK[:, :])
            nc.sync.dma_start(bias_sb[:, 2, :], BV[:, :])

            # ---------------- phase 1: projections ----------------
            with tc.tile_pool(name="projw", bufs=1) as wpool, tc.tile_pool(
                name="xin", bufs=3
            ) as xin:
                w_sb = wpool.tile([P, 3, NKT, DH], f32r)  # wq, wk, wv tiles
                for wi, W in enumerate((WQT, WKT, WVT)):
                    for kt in range(NKT):
                        nc.sync.dma_start(
                            w_sb[:, wi, kt, :], W[kt * P : (kt + 1) * P, :]
                        )

                vT_tmp = wpool.tile([P, 2, S], f32)  # v^T before transpose

                for wi, (X, dst) in enumerate(((QT, qT), (KT, kT), (VT, vT_tmp))):
                    for ihalf in range(2):
                        ps = [
                            psum.tile([P, 1024], f32, tag="nat", name=f"ps{m}")
                            for m in range(2)
                        ]
                        for kt in range(NKT):
                            xt = xin.tile([P, 1024], f32r, tag="xin")
                            nc.sync.dma_start(
                                xt,
                                X[
                                    kt * P : (kt + 1) * P,
                                    ihalf * 1024 : (ihalf + 1) * 1024,
                                ],
                            )
                            for m in range(2):
                                for ib in range(2):
                                    mm(
                                        ps[m][:, ib * 512 : (ib + 1) * 512],
                                        w_sb[:, wi, kt, m * P : (m + 1) * P],
                                        xt[:, ib * 512 : (ib + 1) * 512],
                                        start=(kt == 0),
                                        stop=False,
                                    )
                        for m in range(2):
                            for ib in range(2):
                                mm(
                                    ps[m][:, ib * 512 : (ib + 1) * 512],
                                    bias_sb[0:1, wi, m * P : (m + 1) * P],
                                    ones_row[0:1, :],
                                    start=False,
                                    stop=True,
                                )
                            nc.vector.tensor_copy(
                                dst[:, m, ihalf * 1024 : (ihalf + 1) * 1024],
                                ps[m][:, :],
                            )

                # v^T -> v natural via PE transposes
                for m in range(2):
                    for jc in range(JT):
                        tp = psum.tile([P, P], f32, tag="st")
                        nc.tensor.transpose(
                            tp, vT_tmp[:, m, jc * P : (jc + 1) * P], identity
                        )
                        nc.vector.tensor_copy(
                            v_sb[:, jc, m * P : (m + 1) * P], tp
                        )

            # ---------------- phase 2/3: attention ----------------
            with tc.tile_pool(name="enat", bufs=3) as enat, tc.tile_pool(
                name="pnat", bufs=3
            ) as pnat, tc.tile_pool(name="etp", bufs=4) as etp, tc.tile_pool(
                name="rbc", bufs=3
            ) as rbc_pool, tc.tile_pool(name="smallp", bufs=4) as smallp, tc.tile_pool(
                name="outp", bufs=3
            ) as outp:
                rbc = {}
                for pair in range(2):
                    qTp = qT[:, pair, :]
                    kTp = kT[:, pair, :]
                    # ---- natural-layout scores + softmax + ATTN output ----
                    for e in range(2):
                        h = 2 * pair + e
                        lo, hi = e * DK, (e + 1) * DK
                        for it in range(IT):
                            e_nat = enat.tile([P, S], f32, tag="enat")
                            sums = smallp.tile([P, 2], f32, tag="sums")
                            for jh in range(2):
                                psn = psum.tile([P, 1024], f32, tag="nat")
                                for jb in range(2):
                                    j0 = jh * 1024 + jb * 512
                                    mm(
                                        psn[:, jb * 512 : (jb + 1) * 512],
                                        qTp[lo:hi, it * P : (it + 1) * P],
                                        kTp[lo:hi, j0 : j0 + 512],
                                        start=True,
                                        stop=True,
                                    )
                                nc.scalar.activation(
                                    e_nat[:, jh * 1024 : (jh + 1) * 1024],
                                    psn[:, :],
                                    mybir.ActivationFunctionType.Exp,
                                    accum_out=sums[:, jh : jh + 1],
                                )
                            stot = smallp.tile([P, 1], f32, tag="stot")
                            nc.vector.tensor_reduce(
                                stot,
                                sums,
                                axis=mybir.AxisListType.X,
                                op=mybir.AluOpType.add,
                            )
                            nc.vector.reciprocal(r_mat[:, h, it : it + 1], stot)
                            p_t = pnat.tile([P, S], f32, tag="pnat")
                            nc.vector.tensor_scalar_mul(
                                p_t, e_nat, r_mat[:, h, it : it + 1]
                            )
                            nc.sync.dma_start(
                                ATTN[h, it * P : (it + 1) * P, :], p_t
                            )
                        # r broadcast: r_mat[:,h,:] -> dram (transposed) -> all partitions
                        rt_d = dram.tile([S], f32)
                        rt_write = bass.AP(
                            tensor=rt_d.tensor,
                            offset=rt_d.offset,
                            ap=[[1, P], [P, IT]],
                        )
                        with nc.allow_non_contiguous_dma(
                            reason="2k-element transposed scatter of row sums"
                        ):
                            nc.gpsimd.dma_start(rt_write, r_mat[:, h, :])
                        rbc[h] = rbc_pool.tile(
                            [P, S], f32, tag="rbc", name=f"rbc{h}"
                        )
                        rt_bcast = bass.AP(
                            tensor=rt_d.tensor,
                            offset=rt_d.offset,
                            ap=[[0, P], [1, S]],
                        )
                        nc.gpsimd.dma_start(rbc[h], rt_bcast)

                    # ---- transposed scores + AV ----
                    for iq in range(4):
                        avs = [
                            psum.tile(
                                [DK, 512], f32, tag=f"av{e}", name=f"av{e}", bufs=1
                            )
                            for e in range(2)
                        ]
                        for jt in range(JT):
                            for e in range(2):
                                lo, hi = e * DK, (e + 1) * DK
                                st = psum.tile([P, 512], f32, tag="st")
                                mm(
                                    st,
                                    kTp[lo:hi, jt * P : (jt + 1) * P],
                                    qTp[lo:hi, iq * 512 : (iq + 1) * 512],
                                    start=True,
                                    stop=True,
                                )
                                et = etp.tile([P, 512], f32r, tag="etp")
                                nc.scalar.activation(
                                    et, st, mybir.ActivationFunctionType.Exp
                                )
                                mm(
                                    avs[e],
                                    v_sb[:, jt, pair * P + lo : pair * P + hi],
                                    et,
                                    start=(jt == 0),
                                    stop=(jt == JT - 1),
                                )
                        for e in range(2):
                            h = 2 * pair + e
                            nc.vector.tensor_tensor(
                                o_sb[:, h, iq * 512 : (iq + 1) * 512],
                                avs[e],
                                rbc[h][0:DK, iq * 512 : (iq + 1) * 512],
                                mybir.AluOpType.mult,
                            )

                # ---------------- phase 4: output projection ----------------
                for it in range(IT):
                    out_t = outp.tile([P, D], f32, tag="outp")
                    for nb in range(2):
                        po = psum.tile([P, 512], f32, tag="st")
                        for h in range(HC):
                            mm(
                                po,
                                o_sb[:, h, it * P : (it + 1) * P],
                                wot_sb[:, h, nb * 512 : (nb + 1) * 512],
                                start=(h == 0),
                                stop=(h == HC - 1),
                            )
                        nc.vector.tensor_copy(
                            out_t[:, nb * 512 : (nb + 1) * 512], po
                        )
                    nc.sync.dma_start(OUTP[it * P : (it + 1) * P, :], out_t)

    nc.compile()
    return nc


def _get_nc():
    if "nc" not in _cached:
        _cached["nc"] = _build()
    return _cached["nc"]


def _prep_in_maps(Q, K, V, Wq, bq, Wk, bk, Wv, bv, Wo, bo):
    f4 = np.float32
    Q = np.asarray(Q, f4)
    K = np.asarray(K, f4)
    V = np.asarray(V, f4)
    Wq = np.asarray(Wq, f4)
    Wk = np.asarray(Wk, f4)
    Wv = np.asarray(Wv, f4)
    Wo = np.asarray(Wo, f4)
    bq = np.asarray(bq, f4)
    bk = np.asarray(bk, f4)
    bv = np.asarray(bv, f4)
    bo = np.asarray(bo, f4)

    scale = f4(1.0 / np.sqrt(DK))
    QTb = [np.ascontiguousarray(Q[b].T) for b in range(B)]
    KTb = [np.ascontiguousarray(K[b].T) for b in range(B)]
    VTb = [np.ascontiguousarray(V[b].T) for b in range(B)]

    in_maps = []
    for c in range(N_CORES):
        b, g = c // 4, c % 4
        rows = slice(g * HC * DK, (g + 1) * HC * DK)
        in_maps.append(
            {
                "qt_in": QTb[b],
                "kt_in": KTb[b],
                "vt_in": VTb[b],
                "wqt": np.ascontiguousarray(Wq[rows].T) * scale,
                "wkt": np.ascontiguousarray(Wk[rows].T),
                "wvt": np.ascontiguousarray(Wv[rows].T),
                "wot": np.ascontiguousarray(Wo[:, rows].T),
                "bq_in": (bq[rows] * scale).reshape(1, -1),
                "bk_in": bk[rows].reshape(1, -1),
                "bv_in": bv[rows].reshape(1, -1),
                "ones_in": np.ones((1, 512), f4),
            }
        )
    return in_maps, bo


def _assemble(results, bo):
    f4 = np.float32
    attn = np.empty((B, H, S, S), f4)
    out = np.tile(np.broadcast_to(bo, (S, D)).astype(f4)[None], (B, 1, 1))
    for c in range(N_CORES):
        b, g = c // 4, c % 4
        attn[b, g * HC : (g + 1) * HC] = results[c]["attn_out"]
        out[b] += results[c]["out_partial"]
    return (out, attn)


def kernel(Q, K, V, Wq, bq, Wk, bk, Wv, bv, Wo, bo):
    from concourse import bass_utils

    in_maps, bo = _prep_in_maps(Q, K, V, Wq, bq, Wk, bk, Wv, bv, Wo, bo)
    nc = _get_nc()
    res = bass_utils.run_bass_kernel_spmd(
        nc, in_maps, core_ids=list(range(N_CORES))
    )
    _cached["last_results"] = res
    return _assemble(res.results, bo)
